# revision 13
# baseline (speedup 1.0000x reference)
"""AFM layer kernel for 8 TRN2 NeuronCores.

Math: the reference's attention softmax is over a size-1 axis, so the
attention weights are exactly 1.0 and the attention MLP (Wa, ba, Wh, bh)
cancels out of the output.  What remains is

    pooled[b, :] = sum_{i<j} e_i * e_j          (elementwise over k=16)
                 = 0.5 * ((sum_f e_f)^2 - sum_f e_f^2)
    out[b]       = sigmoid(pooled @ Wo + bo)

where e_f = emb_tables[f, sparse[b, f], :].  The device kernel is an
embedding gather (indirect DMA, one row per partition per instruction —
the only indirect-DMA shape this toolchain lowers correctly) plus a
small amount of vector math.

Sharding: data-parallel over batch; each of the 8 cores handles 256 rows
(2 half-tiles of 128 partition rows, batch row = h*128 + p).  Embedding
tables are replicated; Wo/bo are tiny and replicated.
"""

import numpy as np

try:
    import concourse  # noqa: F401
except ImportError:  # pragma: no cover
    import sys

    sys.path.insert(0, "/opt/trn_rl_repo")

N_FIELDS = 26
VOCAB = 10000
K = 16
BATCH = 2048
N_CORES = 8
PER_CORE = BATCH // N_CORES  # 256
HALVES = PER_CORE // 128  # 2
N_CHUNK = HALVES * N_FIELDS  # 52 gathered rows per partition

_NC_CACHE = {}


def _build_nc():
    from concourse import bass, mybir

    f32 = mybir.dt.float32
    i32 = mybir.dt.int32

    nc = bass.Bass()
    idx_d = nc.declare_dram_parameter("idx", [128, N_CHUNK], i32, isOutput=False)
    emb_d = nc.declare_dram_parameter("emb", [N_FIELDS * VOCAB, K], f32, isOutput=False)
    # cst: Wo broadcast [128,16] ++ bo [128,1] ++ zeros [128,1]
    cst_d = nc.declare_dram_parameter("cst", [128, K + 2], f32, isOutput=False)
    # out layout: [p, h] — batch row h*128 + p lives at out[p, h]
    out_d = nc.declare_dram_parameter("out", [128, HALVES], f32, isOutput=True)

    with (
        nc.sbuf_tensor([128, N_CHUNK], i32) as idx_t,
        nc.sbuf_tensor([128, K + 2], f32) as cst_t,
        nc.sbuf_tensor([128, N_CHUNK * K], f32) as e_t,
        nc.sbuf_tensor([128, N_CHUNK * K], f32) as sq_t,
        nc.sbuf_tensor([128, HALVES * K], f32) as s_t,
        nc.sbuf_tensor([128, HALVES * K], f32) as q_t,
        nc.sbuf_tensor([128, HALVES * K], f32) as sw_t,
        nc.sbuf_tensor([128, HALVES * K], f32) as ssw_t,
        nc.sbuf_tensor([128, HALVES * K], f32) as qw_t,
        nc.sbuf_tensor([128, HALVES], f32) as t_acc,
        nc.sbuf_tensor([128, HALVES], f32) as u_acc,
        nc.sbuf_tensor([128, HALVES], f32) as d_t,
        nc.sbuf_tensor([128, HALVES], f32) as y_t,
        nc.semaphore("c_sem") as c_sem,
        nc.semaphore("i_sem") as i_sem,
        nc.semaphore("g_sem") as g_sem,
        nc.semaphore("v_sem") as v_sem,
        nc.semaphore("a_sem") as a_sem,
        nc.semaphore("o_sem") as o_sem,
        nc.Block(no_gpsimd_drain=True) as block,
    ):
        wo_v = cst_t[:, 0:K]
        bo_v = cst_t[:, K : K + 1]
        zero_v = cst_t[:, K + 1 : K + 2]

        # e_t free layout per partition: [h, f, k]
        e_hfk = e_t[:, :].rearrange("p (h f k) -> p h f k", h=HALVES, f=N_FIELDS, k=K)
        e_hkf = e_hfk.transpose([0, 1, 3, 2])
        sq_hfk = sq_t[:, :].rearrange("p (h f k) -> p h f k", h=HALVES, f=N_FIELDS, k=K)
        sq_hkf = sq_hfk.transpose([0, 1, 3, 2])
        s_v = s_t[:, :].rearrange("p (h k) -> p h k", h=HALVES, k=K)
        q_v = q_t[:, :].rearrange("p (h k) -> p h k", h=HALVES, k=K)
        sw_v = sw_t[:, :].rearrange("p (h k) -> p h k", h=HALVES, k=K)
        ssw_v = ssw_t[:, :].rearrange("p (h k) -> p h k", h=HALVES, k=K)
        qw_v = qw_t[:, :].rearrange("p (h k) -> p h k", h=HALVES, k=K)
        t_v = t_acc[:, :].rearrange("p (h o) -> p h o", h=HALVES, o=1)
        u_v = u_acc[:, :].rearrange("p (h o) -> p h o", h=HALVES, o=1)

        @block.sync
        def _(sp):
            sp.dma_start(out=idx_t[:, :], in_=idx_d[:, :]).then_inc(i_sem, 16)
            sp.dma_start(out=cst_t[:, :], in_=cst_d[:, :]).then_inc(c_sem, 16)
            # wait for the sigmoid (3rd ACT inc) then store output
            sp.wait_ge(a_sem, 3)
            sp.dma_start(out=out_d[:, :], in_=y_t[:, :]).then_inc(o_sem, 16)
            sp.wait_ge(o_sem, 16)

        @block.gpsimd
        def _(g):
            g.wait_ge(i_sem, 16)
            for j in range(N_CHUNK):
                inst = g.indirect_dma_start(
                    out=e_t[:, j * K : (j + 1) * K],
                    out_offset=None,
                    in_=emb_d[:, :],
                    in_offset=bass.IndirectOffsetOnAxis(
                        ap=idx_t[:, j : j + 1], axis=0
                    ),
                )
                inst.then_inc(g_sem, 16)

        # NOTE on hazards (empirically established on this toolchain):
        # - a DVE/ACT instruction reading an SBUF region written by one of
        #   its ~2 immediately preceding same-engine instructions sees stale
        #   data (no HW interlock; engine_nop does not help — real ops do);
        # - a cross-engine consumer gated only by .then_inc on the producing
        #   instruction can also see stale data, so handoff sem incs ride on
        #   a drain preceded by >=2 unrelated real ops.
        @block.scalar
        def _(s):
            s.wait_ge(c_sem, 16)  # zero bias + bo available
            for h in range(HALVES):
                s.wait_ge(g_sem, 16 * N_FIELDS * (h + 1))
                s.activation(
                    sq_hfk[:, h],
                    e_hfk[:, h],
                    func=mybir.ActivationFunctionType.Square,
                    bias=zero_v,
                    scale=1.0,
                )
                s.drain().then_inc(a_sem, 1)
            s.wait_ge(v_sem, 1)
            s.activation(
                y_t[:, :],
                d_t[:, :],
                func=mybir.ActivationFunctionType.Sigmoid,
                bias=bo_v,
                scale=0.5,
            )
            # spacer ops so y_t's write lands before the drain's sem inc
            s.activation(
                sq_hfk[:, 0, 0:1, :],
                e_hfk[:, 0, 0:1, :],
                func=mybir.ActivationFunctionType.Square,
                bias=zero_v,
                scale=1.0,
            )
            s.activation(
                sq_hfk[:, 0, 1:2, :],
                e_hfk[:, 0, 1:2, :],
                func=mybir.ActivationFunctionType.Square,
                bias=zero_v,
                scale=1.0,
            )
            s.drain().then_inc(a_sem, 1)

        @block.vector
        def _(v):
            v.wait_ge(c_sem, 16)  # wo available
            v.wait_ge(g_sem, 16 * N_FIELDS)
            v.reduce_sum(s_v[:, 0], e_hkf[:, 0], axis=mybir.AxisListType.X)
            v.wait_ge(a_sem, 1)
            v.reduce_sum(q_v[:, 0], sq_hkf[:, 0], axis=mybir.AxisListType.X)
            v.wait_ge(g_sem, 16 * N_FIELDS * 2)
            v.reduce_sum(s_v[:, 1], e_hkf[:, 1], axis=mybir.AxisListType.X)
            v.wait_ge(a_sem, 2)
            v.reduce_sum(q_v[:, 1], sq_hkf[:, 1], axis=mybir.AxisListType.X)
            v.tensor_mul(out=sw_v[:, 0], in0=s_v[:, 0], in1=wo_v)
            v.tensor_mul(out=sw_v[:, 1], in0=s_v[:, 1], in1=wo_v)
            v.tensor_mul(out=qw_v[:, 0], in0=q_v[:, 0], in1=wo_v)
            v.tensor_mul(out=qw_v[:, 1], in0=q_v[:, 1], in1=wo_v)
            v.tensor_mul(out=ssw_v[:, 0], in0=s_v[:, 0], in1=sw_v[:, 0])
            v.tensor_mul(out=ssw_v[:, 1], in0=s_v[:, 1], in1=sw_v[:, 1])
            v.reduce_sum(u_v[:, 0], qw_v[:, 0], axis=mybir.AxisListType.X)
            v.reduce_sum(u_v[:, 1], qw_v[:, 1], axis=mybir.AxisListType.X)
            v.reduce_sum(t_v[:, 0], ssw_v[:, 0], axis=mybir.AxisListType.X)
            v.reduce_sum(t_v[:, 1], ssw_v[:, 1], axis=mybir.AxisListType.X)
            # spacers so t_acc's writes land before the sub reads them
            v.tensor_mul(out=qw_v[:, 0], in0=q_v[:, 0], in1=wo_v)
            v.tensor_mul(out=qw_v[:, 1], in0=q_v[:, 1], in1=wo_v)
            v.tensor_sub(out=d_t[:, :], in0=t_acc[:, :], in1=u_acc[:, :])
            # spacers so d_t's write lands before the drain's sem inc
            v.tensor_mul(out=sw_v[:, 0], in0=s_v[:, 0], in1=wo_v)
            v.tensor_mul(out=sw_v[:, 1], in0=s_v[:, 1], in1=wo_v)
            v.drain().then_inc(v_sem, 1)

    return nc


def _get_nc():
    if "nc" not in _NC_CACHE:
        _NC_CACHE["nc"] = _build_nc()
    return _NC_CACHE["nc"]


def _prep_in_maps(sparse, emb_tables, Wo, bo):
    sparse = np.asarray(sparse)
    emb_flat = np.ascontiguousarray(
        np.asarray(emb_tables, dtype=np.float32).reshape(N_FIELDS * VOCAB, K)
    )

    # flat row index into the stacked [26*10000, 16] table
    flat_idx = (
        sparse.astype(np.int32) + (np.arange(N_FIELDS, dtype=np.int32) * VOCAB)[None, :]
    )  # [2048, 26]

    cst = np.zeros((128, K + 2), dtype=np.float32)
    cst[:, 0:K] = np.asarray(Wo, dtype=np.float32).reshape(1, K)
    cst[:, K] = np.float32(np.asarray(bo).reshape(-1)[0])

    in_maps = []
    for c in range(N_CORES):
        rows = flat_idx[c * PER_CORE : (c + 1) * PER_CORE]  # [256, 26]
        # [h, p, f] -> [p, (h f)]
        idx_c = np.ascontiguousarray(
            rows.reshape(HALVES, 128, N_FIELDS).transpose(1, 0, 2).reshape(128, N_CHUNK)
        )
        in_maps.append({"idx": idx_c, "emb": emb_flat, "cst": cst})
    return in_maps


def _run(in_maps, trace=False, **kwargs):
    from concourse.bass_utils import run_bass_kernel_spmd

    nc = _get_nc()
    return run_bass_kernel_spmd(
        nc, in_maps, core_ids=list(range(N_CORES)), trace=trace, **kwargs
    )


def _collect_out(res):
    # res out[c] is [128, HALVES]; batch row c*256 + h*128 + p = out[c][p, h]
    return np.concatenate(
        [res.results[c]["out"].T.reshape(PER_CORE, 1) for c in range(N_CORES)], axis=0
    ).astype(np.float32)


def kernel(dense, sparse, emb_tables, Wa, ba, Wh, bh, Wo, bo):
    in_maps = _prep_in_maps(sparse, emb_tables, Wo, bo)
    res = _run(in_maps)
    return _collect_out(res)
